# revision 1
# baseline (speedup 1.0000x reference)
"""Trainium2 Bass kernel for a custom LSTM cell.

Math (per reference):
    i = sigmoid(x @ W_i.T + b_Wi + h @ U_i.T + b_Ui)
    f = sigmoid(x @ W_f.T + b_Wf + h @ U_f.T + b_Uf + boundary @ W_b.T + b_Wb)
    o = sigmoid(x @ W_o.T + b_Wo + h @ U_o.T + b_Uo)
    g = tanh   (x @ W_g.T + b_Wg + h @ U_g.T + b_Ug)
    c = f * c_prev + i * g
    h = o * tanh(c)

Strategy: data-parallel over batch across 8 NeuronCores (1024 rows each).
Host-side we build A.T = [x | h_prev].T (K=1536 on partitions) and a single
fused weight matrix M [1536, 4096] whose columns are ordered per 256-wide
h-slice as [i | f | o | g], so the device only does natural-layout DMAs and
K-partition matmuls. Bias + boundary enter as one extra K=3 matmul step
(lhsT rows = [ones, boundary0, boundary1]). Matmuls run in float32r (TF32)
at full PE rate; operands are rounded to f32r by DVE copies after fast
HWDGE loads.
"""

import sys

sys.path.insert(0, "/opt/trn_rl_repo")

import numpy as np

B, IN, H = 8192, 512, 1024
NCORES = 8
BLOC = B // NCORES  # 1024 rows per core
KTOT = IN + H  # 1536 contraction
KT = KTOT // 128  # 12 k-tiles
BT = BLOC // 128  # 8 batch tiles per core
SLICE = 256  # h-slice width per gate
NS = H // SLICE  # 4 slices
GW = 4 * SLICE  # 1024 columns of M per slice (i|f|o|g)

_PROG = None  # cached so repeat calls skip rebuild/recompile


def _build_program():
    import concourse.bass as bass
    import concourse.mybir as mybir
    import concourse.tile as tile
    from concourse import bacc
    from contextlib import ExitStack

    f32 = mybir.dt.float32
    f32r = mybir.dt.float32r
    bf16 = mybir.dt.bfloat16
    SIG = mybir.ActivationFunctionType.Sigmoid
    TANH = mybir.ActivationFunctionType.Tanh

    nc = bacc.Bacc("TRN2", target_bir_lowering=False, debug=False)

    at_d = nc.dram_tensor("at_in", [KTOT, BLOC], f32r, kind="ExternalInput").ap()
    et_d = nc.dram_tensor("et_in", [3, BLOC], f32r, kind="ExternalInput").ap()
    m_d = nc.dram_tensor("m_in", [KTOT, 4 * H], f32r, kind="ExternalInput").ap()
    r_d = nc.dram_tensor("r_in", [3, 4 * H], f32r, kind="ExternalInput").ap()
    c_d = nc.dram_tensor("c_in", [BLOC, H], f32, kind="ExternalInput").ap()
    h_o = nc.dram_tensor("h_out", [BLOC, H], f32, kind="ExternalOutput").ap()
    c_o = nc.dram_tensor("c_out", [BLOC, H], f32, kind="ExternalOutput").ap()

    with tile.TileContext(nc) as tc:
        with ExitStack() as ctx:
            atp = ctx.enter_context(tc.tile_pool(name="atp", bufs=1))
            mp = ctx.enter_context(tc.tile_pool(name="mp", bufs=2))
            cst = ctx.enter_context(tc.tile_pool(name="cst", bufs=1))
            cinp = ctx.enter_context(tc.tile_pool(name="cinp", bufs=4))
            actp = ctx.enter_context(tc.tile_pool(name="actp", bufs=2))
            outp = ctx.enter_context(tc.tile_pool(name="outp", bufs=4))
            psp = ctx.enter_context(tc.tile_pool(name="psp", bufs=8, space="PSUM"))
            wup = ctx.enter_context(tc.tile_pool(name="wup", bufs=1))

            # PE warm-up: dummy bf16 matmuls with no DMA deps keep the PE HAM
            # clock gate busy while the first weight tiles load.
            wu_w = wup.tile([128, 128], bf16, name="wu_w")
            nc.vector.memset(wu_w, 0.0)
            wu_ps = psp.tile([128, 512], f32, name="wu_ps", tag="ps")
            for _ in range(72):
                nc.tensor.matmul(wu_ps[:, 0:128], wu_w, wu_w, start=True, stop=True)

            et_t = cst.tile([3, BLOC], f32r, name="et_t")
            nc.sync.dma_start(out=et_t, in_=et_d[:, :])
            r_t = cst.tile([3, 4 * H], f32r, name="r_t")
            nc.sync.dma_start(out=r_t, in_=r_d[:, :])

            def load_m_slice(s):
                """One [128, 12, GW] tile per slice, filled by 3 big 3D DMAs."""
                t = mp.tile([128, KT, GW], f32r, name=f"m_{s}", tag="m")
                for j in range(3):
                    nc.sync.dma_start(
                        out=t[:, j * 4 : (j + 1) * 4, :],
                        in_=m_d[
                            j * 512 : (j + 1) * 512, s * GW : (s + 1) * GW
                        ].rearrange("(kk p) g -> p kk g", p=128),
                    )
                return t

            # AT interleaved with slice-0 weights so matmuls start early
            at_t = atp.tile([128, KT, BLOC], f32r, name="at_t")
            m_t = mp.tile([128, KT, GW], f32r, name="m_0", tag="m")
            for j in range(3):
                nc.sync.dma_start(
                    out=at_t[:, j * 4 : (j + 1) * 4, :],
                    in_=at_d[j * 512 : (j + 1) * 512, :].rearrange(
                        "(kk p) g -> p kk g", p=128
                    ),
                )
                nc.sync.dma_start(
                    out=m_t[:, j * 4 : (j + 1) * 4, :],
                    in_=m_d[j * 512 : (j + 1) * 512, 0:GW].rearrange(
                        "(kk p) g -> p kk g", p=128
                    ),
                )

            for s in range(NS):
                if s > 0:
                    m_t = load_m_slice(s)

                for b in range(BT):
                    bs = slice(b * 128, (b + 1) * 128)
                    ps_if = psp.tile([128, 512], f32, name=f"psif{s}_{b}", tag="ps")
                    ps_og = psp.tile([128, 512], f32, name=f"psog{s}_{b}", tag="ps")
                    for k in range(KT):
                        lhs = at_t[:, k, bs]
                        nc.tensor.matmul(
                            ps_if,
                            lhs,
                            m_t[:, k, 0:512],
                            start=(k == 0),
                            stop=False,
                        )
                        nc.tensor.matmul(
                            ps_og,
                            lhs,
                            m_t[:, k, 512:1024],
                            start=(k == 0),
                            stop=False,
                        )
                    # bias + boundary: K=3 step, rows [ones, bdry0, bdry1]
                    elhs = et_t[:, bs]
                    nc.tensor.matmul(
                        ps_if,
                        elhs,
                        r_t[:, s * GW : s * GW + 512],
                        start=False,
                        stop=True,
                    )
                    nc.tensor.matmul(
                        ps_og,
                        elhs,
                        r_t[:, s * GW + 512 : (s + 1) * GW],
                        start=False,
                        stop=True,
                    )

                    # gate nonlinearities (i,f -> sigmoid; o -> sigmoid; g -> tanh)
                    if_t = actp.tile([128, 512], f32, name=f"if{s}_{b}", tag="if")
                    og_t = actp.tile([128, 512], f32, name=f"og{s}_{b}", tag="og")
                    nc.scalar.activation(if_t, ps_if, SIG)
                    nc.scalar.activation(og_t[:, 0:SLICE], ps_og[:, 0:SLICE], SIG)
                    nc.scalar.activation(og_t[:, SLICE:512], ps_og[:, SLICE:512], TANH)

                    c_t = cinp.tile([128, SLICE], f32, name=f"cin{s}_{b}", tag="cin")
                    nc.scalar.dma_start(
                        out=c_t, in_=c_d[bs, s * SLICE : (s + 1) * SLICE]
                    )

                    cn = outp.tile([128, SLICE], f32, name=f"cn{s}_{b}", tag="cn")
                    tmp = actp.tile([128, SLICE], f32, name=f"tmp{s}_{b}", tag="tmp")
                    # c' = f*c_prev + i*g
                    nc.vector.tensor_mul(cn, if_t[:, SLICE:512], c_t)
                    nc.vector.tensor_mul(tmp, if_t[:, 0:SLICE], og_t[:, SLICE:512])
                    nc.vector.tensor_add(cn, cn, tmp)
                    th = actp.tile([128, SLICE], f32, name=f"th{s}_{b}", tag="th")
                    nc.scalar.activation(th, cn, TANH)
                    hn = outp.tile([128, SLICE], f32, name=f"hn{s}_{b}", tag="hn")
                    nc.vector.tensor_mul(hn, og_t[:, 0:SLICE], th)

                    nc.scalar.dma_start(
                        out=c_o[bs, s * SLICE : (s + 1) * SLICE], in_=cn
                    )
                    nc.scalar.dma_start(
                        out=h_o[bs, s * SLICE : (s + 1) * SLICE], in_=hn
                    )
    nc.compile()
    return nc


def _get_program():
    global _PROG
    if _PROG is None:
        _PROG = _build_program()
    return _PROG


def _tf32(a):
    """Round float32 ndarray to TF32 (10-bit mantissa, RNE)."""
    b = np.ascontiguousarray(a, np.float32).view(np.uint32)
    lsb = (b >> np.uint32(13)) & np.uint32(1)
    r = (b + np.uint32(0x0FFF) + lsb) & ~np.uint32(0x1FFF)
    return r.view(np.float32)


def _prep_inputs(inputs):
    """Host-side marshalling: fused weight matrix + transposed activations."""
    f = np.float32
    x = np.asarray(inputs["x"], f)
    h_prev = np.asarray(inputs["h_prev"], f)
    c_prev = np.asarray(inputs["c_prev"], f)
    boundary = np.asarray(inputs["boundary"], f)

    gates = ["i", "f", "o", "g"]
    W = {z: np.asarray(inputs[f"W_{z}"], f) for z in gates}
    U = {z: np.asarray(inputs[f"U_{z}"], f) for z in gates}
    bias = {
        z: np.asarray(inputs[f"b_W{z}"], f) + np.asarray(inputs[f"b_U{z}"], f)
        for z in gates
    }
    W_b = np.asarray(inputs["W_b"], f)
    b_Wb = np.asarray(inputs["b_Wb"], f)

    # M [1536, 4096]: rows 0-511 W.T, rows 512-1535 U.T; columns ordered per
    # 256-wide h-slice as [i | f | o | g].
    M = np.empty((KTOT, 4 * H), f)
    R = np.zeros((3, 4 * H), f)  # row0 bias; rows 1-2 boundary weights (f only)
    for s in range(NS):
        hs = slice(s * SLICE, (s + 1) * SLICE)
        for zi, z in enumerate(gates):
            cs = slice(s * GW + zi * SLICE, s * GW + (zi + 1) * SLICE)
            M[:IN, cs] = W[z][hs].T
            M[IN:, cs] = U[z][hs].T
            R[0, cs] = bias[z][hs]
            if z == "f":
                R[0, cs] += b_Wb[hs]
                R[1:3, cs] = W_b[hs].T

    AT = np.concatenate([x, h_prev], axis=1).T  # [1536, 8192]
    ET = np.concatenate(
        [np.ones((1, B), f), boundary.T.astype(f)], axis=0
    )  # [3, 8192]

    MR = _tf32(M)
    RR = _tf32(R)
    in_maps = []
    for c in range(NCORES):
        rs = slice(c * BLOC, (c + 1) * BLOC)
        in_maps.append(
            {
                "at_in": _tf32(AT[:, rs]),
                "et_in": _tf32(ET[:, rs]),
                "m_in": MR,
                "r_in": RR,
                "c_in": np.ascontiguousarray(c_prev[rs]),
            }
        )
    return in_maps


def run(inputs, trace=False):
    """Returns ((h, c), BassKernelResults)."""
    from concourse.bass_utils import run_bass_kernel_spmd

    nc = _get_program()
    in_maps = _prep_inputs(inputs)
    res = run_bass_kernel_spmd(
        nc, in_maps, core_ids=list(range(NCORES)), trace=trace
    )
    h = np.concatenate([r["h_out"] for r in res.results], axis=0)
    c = np.concatenate([r["c_out"] for r in res.results], axis=0)
    return (h, c), res


def kernel(**inputs):
    out, _ = run(inputs, trace=False)
    return out



# revision 3
# speedup vs baseline: 1.3931x; 1.3931x over previous
"""Trainium2 Bass kernel for a custom LSTM cell.

Math (per reference):
    i = sigmoid(x @ W_i.T + b_Wi + h @ U_i.T + b_Ui)
    f = sigmoid(x @ W_f.T + b_Wf + h @ U_f.T + b_Uf + boundary @ W_b.T + b_Wb)
    o = sigmoid(x @ W_o.T + b_Wo + h @ U_o.T + b_Uo)
    g = tanh   (x @ W_g.T + b_Wg + h @ U_g.T + b_Ug)
    c = f * c_prev + i * g
    h = o * tanh(c)

Strategy: data-parallel over batch across 8 NeuronCores (1024 rows each).
Unlike the batch-on-partitions baseline, PSUM output tiles put GATE dims on
partitions and batch on the free axis (out = M_tile.T @ A_tile).  That lets
the per-gate bias ride the scalar-engine activation instruction (per-partition
bias + scale), removing all dedicated bias matmuls from the PE.

Operands are bf16 (same PE rate as f32r, half the LDWEIGHTS and DMA cost).
The last NFP8 of the 8 h-contraction subtiles run as fp8e4m3 DoubleRow
matmuls (2 K-subtiles per instruction, 2x PE throughput).  All matmul
operands on the weight side are pre-scaled by 128 on the host so the fp8
U-weights sit in e4m3's normal range; the activation instruction's
scale=1/128 undoes it before sigmoid/tanh.

Host marshalling pre-arranges every tensor into the exact SBUF layout
(partition-major), so all DMAs are contiguous per partition.
"""

import sys

sys.path.insert(0, "/opt/trn_rl_repo")

import numpy as np
import ml_dtypes

B, IN, H = 8192, 512, 1024
NCORES = 8
BLOC = B // NCORES  # 1024 batch rows per core
KTOT = IN + H  # 1536 contraction
KT = KTOT // 128  # 12 K-subtiles total
NFP8 = 4  # of the 8 h-side K-subtiles, how many run fp8 DoubleRow (even)
KBF = KT - NFP8  # leading bf16 K-subtiles (x part + first h subtiles)
NG = 4 * H // 128  # 32 gate-tiles of 128 output dims
NS = H // 128  # 8 h-slices
NCH = BLOC // 512  # 2 batch chunks of 512
SCALE = 128.0  # weight pre-scale (power of two; undone in activation)

_PROG = None  # cached so repeat calls skip rebuild/recompile


def _build_program():
    import concourse.bass as bass
    import concourse.mybir as mybir
    import concourse.tile as tile
    from concourse import bacc
    from contextlib import ExitStack

    f32 = mybir.dt.float32
    bf16 = mybir.dt.bfloat16
    f8 = mybir.dt.float8e4
    SIG = mybir.ActivationFunctionType.Sigmoid
    TANH = mybir.ActivationFunctionType.Tanh
    DR = mybir.MatmulPerfMode.DoubleRow

    nc = bacc.Bacc("TRN2", target_bir_lowering=False, debug=False)

    # Device tensors, all in final SBUF layout (partition dim first).
    # m_bf: [128, NG, KBF, 128] bf16   weights, K-subtiles 0..KBF-1
    # m_f8: [128, NG, NFP8, 128] fp8   weights, K-subtiles KBF..11
    # at_bf: [128, KBF, BLOC] bf16     activations [x | h_lo]
    # at_f8: [128, NFP8, BLOC] fp8     activations h_hi
    # bias: [128, NG] f32              per gate-tile per partition
    # bd:   [2, BLOC] bf16             boundary.T
    # wb:   [2, H] bf16                W_b.T * SCALE
    # c_in: [128, NS, BLOC] f32        c_prev.T
    # h/c out: [128, NS, BLOC] f32     transposed outputs
    m_bf_d = nc.dram_tensor("m_bf", [128, NG * KBF * 128], bf16, kind="ExternalInput").ap()
    at_bf_d = nc.dram_tensor("at_bf", [128, KBF * BLOC], bf16, kind="ExternalInput").ap()
    bias_d = nc.dram_tensor("bias_in", [128, NG], f32, kind="ExternalInput").ap()
    bd_d = nc.dram_tensor("bd_in", [2, BLOC], bf16, kind="ExternalInput").ap()
    wb_d = nc.dram_tensor("wb_in", [2, H], bf16, kind="ExternalInput").ap()
    c_d = nc.dram_tensor("c_in", [128, NS * BLOC], f32, kind="ExternalInput").ap()
    h_o = nc.dram_tensor("h_out", [128, NS * BLOC], f32, kind="ExternalOutput").ap()
    c_o = nc.dram_tensor("c_out", [128, NS * BLOC], f32, kind="ExternalOutput").ap()
    if NFP8:
        m_f8_d = nc.dram_tensor(
            "m_f8", [128, NG * NFP8 * 128], f8, kind="ExternalInput"
        ).ap()
        at_f8_d = nc.dram_tensor(
            "at_f8", [128, NFP8 * BLOC], f8, kind="ExternalInput"
        ).ap()

    with tile.TileContext(nc) as tc:
        with ExitStack() as ctx:
            wup = ctx.enter_context(tc.tile_pool(name="wup", bufs=1))
            cst = ctx.enter_context(tc.tile_pool(name="cst", bufs=1))
            wp = ctx.enter_context(tc.tile_pool(name="wp", bufs=1))
            actp = ctx.enter_context(tc.tile_pool(name="actp", bufs=2))
            outp = ctx.enter_context(tc.tile_pool(name="outp", bufs=4))
            psp = ctx.enter_context(tc.tile_pool(name="psp", bufs=8, space="PSUM"))

            # PE warm-up: dummy bf16 matmuls with no DMA deps push the PE
            # through its p-state ramp while the first weights load.
            wu_w = wup.tile([128, 128], bf16, name="wu_w")
            nc.vector.memset(wu_w, 0.0)
            wu_ps = psp.tile([128, 512], f32, name="wu_ps", tag="ps")
            for _ in range(72):
                nc.tensor.matmul(wu_ps[:, 0:128], wu_w, wu_w, start=True, stop=True)

            # Small constants + activations (DVE queue), c_prev (ACT queue).
            bias_t = cst.tile([128, NG], f32, name="bias_t")
            nc.scalar.dma_start(out=bias_t, in_=bias_d[:, :])
            bd_t = cst.tile([2, BLOC], bf16, name="bd_t")
            nc.scalar.dma_start(out=bd_t, in_=bd_d[:, :])
            wb_t = cst.tile([2, H], bf16, name="wb_t")
            nc.scalar.dma_start(out=wb_t, in_=wb_d[:, :])
            at_bf_t = cst.tile([128, KBF, BLOC], bf16, name="at_bf_t")
            nc.scalar.dma_start(
                out=at_bf_t, in_=at_bf_d[:, :].rearrange("p (k b) -> p k b", k=KBF)
            )
            if NFP8:
                at_f8_t = cst.tile([128, NFP8, BLOC], f8, name="at_f8_t")
                nc.scalar.dma_start(
                    out=at_f8_t,
                    in_=at_f8_d[:, :].rearrange("p (k b) -> p k b", k=NFP8),
                )
            c_t = cst.tile([128, NS, BLOC], f32, name="c_t")
            nc.scalar.dma_start(
                out=c_t, in_=c_d[:, :].rearrange("p (s b) -> p s b", s=NS)
            )

            # Weights: one DMA per gate-tile, issued in compute order.
            w_bf = {}
            w_f8 = {}
            gorder = [s + 8 * z for s in range(NS) for z in range(4)]
            for g in gorder:
                t = wp.tile([128, KBF, 128], bf16, name=f"wbf{g}")
                nc.sync.dma_start(
                    out=t,
                    in_=m_bf_d[:, g * KBF * 128 : (g + 1) * KBF * 128].rearrange(
                        "p (k c) -> p k c", k=KBF
                    ),
                )
                w_bf[g] = t
                if NFP8:
                    t8 = wp.tile([128, NFP8, 128], f8, name=f"wf8{g}")
                    nc.sync.dma_start(
                        out=t8,
                        in_=m_f8_d[:, g * NFP8 * 128 : (g + 1) * NFP8 * 128].rearrange(
                            "p (k c) -> p k c", k=NFP8
                        ),
                    )
                    w_f8[g] = t8

            for ch in range(NCH):
                cs = slice(ch * 512, (ch + 1) * 512)
                for s in range(NS):
                    # gate order i, f, o, g -> gate-tile ids s, 8+s, 16+s, 24+s
                    ps = {}
                    for z in range(4):
                        g = 8 * z + s
                        p = psp.tile([128, 512], f32, name=f"ps{ch}_{s}_{z}", tag="ps")
                        ps[z] = p
                        if z == 1:
                            # boundary influence seeds the f-gate accumulator
                            nc.tensor.matmul(
                                p, wb_t[:, s * 128 : (s + 1) * 128], bd_t[:, cs],
                                start=True, stop=False,
                            )
                        for k in range(KBF):
                            nc.tensor.matmul(
                                p, w_bf[g][:, k, :], at_bf_t[:, k, cs],
                                start=(k == 0 and z != 1),
                                stop=(NFP8 == 0 and k == KBF - 1),
                            )
                        for kp in range(0, NFP8, 2):
                            nc.tensor.matmul(
                                p,
                                w_f8[g][:, kp : kp + 2, :],
                                at_f8_t[:, kp : kp + 2, cs],
                                start=False,
                                stop=(kp == NFP8 - 2),
                                perf_mode=DR,
                            )

                    # activations: sigmoid(i,f,o), tanh(g); bias+scale folded in
                    gt = {}
                    for z, fn in ((0, SIG), (1, SIG), (2, SIG), (3, TANH)):
                        g = 8 * z + s
                        t = actp.tile([128, 512], f32, name=f"g{ch}_{s}_{z}", tag=f"g{z}")
                        nc.scalar.activation(
                            t, ps[z], fn, bias=bias_t[:, g : g + 1], scale=1.0 / SCALE
                        )
                        gt[z] = t

                    # c' = f*c + i*g ; h = o*tanh(c')
                    cn = outp.tile([128, 512], f32, name=f"cn{ch}_{s}", tag="cn")
                    tmp = actp.tile([128, 512], f32, name=f"tmp{ch}_{s}", tag="tmp")
                    nc.vector.tensor_mul(cn, gt[1], c_t[:, s, cs])
                    nc.vector.tensor_mul(tmp, gt[0], gt[3])
                    nc.vector.tensor_add(cn, cn, tmp)
                    th = actp.tile([128, 512], f32, name=f"th{ch}_{s}", tag="th")
                    nc.scalar.activation(th, cn, TANH)
                    hn = outp.tile([128, 512], f32, name=f"hn{ch}_{s}", tag="hn")
                    nc.vector.tensor_mul(hn, gt[2], th)

                    off = s * BLOC + ch * 512
                    nc.gpsimd.dma_start(out=c_o[:, off : off + 512], in_=cn)
                    nc.gpsimd.dma_start(out=h_o[:, off : off + 512], in_=hn)
    nc.compile()
    return nc


def _get_program():
    global _PROG
    if _PROG is None:
        _PROG = _build_program()
    return _PROG


def _prep_inputs(inputs):
    """Host-side marshalling into exact SBUF layouts (see header)."""
    f = np.float32
    bf = ml_dtypes.bfloat16
    f8 = ml_dtypes.float8_e4m3
    x = np.asarray(inputs["x"], f)
    h_prev = np.asarray(inputs["h_prev"], f)
    c_prev = np.asarray(inputs["c_prev"], f)
    boundary = np.asarray(inputs["boundary"], f)

    gates = ["i", "f", "o", "g"]
    # M [1536, 4096]: rows = contraction (x then h), cols = [i|f|o|g] x H.
    M = np.empty((KTOT, 4 * H), f)
    bias_vec = np.empty(4 * H, f)
    for zi, z in enumerate(gates):
        W = np.asarray(inputs[f"W_{z}"], f)
        U = np.asarray(inputs[f"U_{z}"], f)
        cs = slice(zi * H, (zi + 1) * H)
        M[:IN, cs] = W.T
        M[IN:, cs] = U.T
        b = np.asarray(inputs[f"b_W{z}"], f) + np.asarray(inputs[f"b_U{z}"], f)
        if z == "f":
            b = b + np.asarray(inputs["b_Wb"], f)
        bias_vec[cs] = b
    M *= SCALE

    # [K, 4H] -> [128, NG, KS, 128]: K-row = 128*kk + p, col = 128*g + c
    def dev_weights(Msub, ks):
        t = Msub.reshape(ks, 128, NG, 128).transpose(1, 2, 0, 3)
        return np.ascontiguousarray(t).reshape(128, -1)

    m_bf = dev_weights(M[: KBF * 128], KBF).astype(bf)
    bias_dev = np.ascontiguousarray(bias_vec.reshape(NG, 128).T)  # [128, NG]
    wb_dev = np.ascontiguousarray(
        (np.asarray(inputs["W_b"], f).T * SCALE).astype(bf)
    )  # [2, H]

    AT = np.concatenate([x, h_prev], axis=1).T  # [1536, 8192] (full batch)
    at_bf_full = np.ascontiguousarray(
        AT[: KBF * 128].reshape(KBF, 128, B).transpose(1, 0, 2)
    ).astype(bf)  # [128, KBF, B]
    cT = c_prev.T  # [H, B]

    in_maps = []
    if NFP8:
        m_f8 = dev_weights(M[KBF * 128 :], NFP8).astype(f8)
        at_f8_full = np.ascontiguousarray(
            AT[KBF * 128 :].reshape(NFP8, 128, B).transpose(1, 0, 2)
        ).astype(f8)  # [128, NFP8, B]
    for c in range(NCORES):
        rs = slice(c * BLOC, (c + 1) * BLOC)
        im = {
            "m_bf": m_bf,
            "at_bf": np.ascontiguousarray(at_bf_full[:, :, rs]).reshape(128, -1),
            "bias_in": bias_dev,
            "bd_in": np.ascontiguousarray(boundary[rs].T.astype(bf)),
            "wb_in": wb_dev,
            "c_in": np.ascontiguousarray(
                cT[:, rs].reshape(NS, 128, BLOC).transpose(1, 0, 2)
            ).reshape(128, -1),
        }
        if NFP8:
            im["m_f8"] = m_f8
            im["at_f8"] = np.ascontiguousarray(at_f8_full[:, :, rs]).reshape(128, -1)
        in_maps.append(im)
    return in_maps


def _unshard(res_list, key):
    """[128, NS*BLOC] per core -> [B, H] full."""
    parts = []
    for r in res_list:
        t = r[key].reshape(128, NS, BLOC).transpose(1, 0, 2).reshape(H, BLOC)
        parts.append(t.T)  # [BLOC, H]
    return np.ascontiguousarray(np.concatenate(parts, axis=0))


def run(inputs, trace=False):
    """Returns ((h, c), BassKernelResults)."""
    from concourse.bass_utils import run_bass_kernel_spmd

    nc = _get_program()
    in_maps = _prep_inputs(inputs)
    res = run_bass_kernel_spmd(
        nc, in_maps, core_ids=list(range(NCORES)), trace=trace
    )
    h = _unshard(res.results, "h_out")
    c = _unshard(res.results, "c_out")
    return (h, c), res


def kernel(**inputs):
    out, _ = run(inputs, trace=False)
    return out


# revision 8
# speedup vs baseline: 1.4588x; 1.0472x over previous
"""Trainium2 Bass kernel for a custom LSTM cell.

Math (per reference):
    i = sigmoid(x @ W_i.T + b_Wi + h @ U_i.T + b_Ui)
    f = sigmoid(x @ W_f.T + b_Wf + h @ U_f.T + b_Uf + boundary @ W_b.T + b_Wb)
    o = sigmoid(x @ W_o.T + b_Wo + h @ U_o.T + b_Uo)
    g = tanh   (x @ W_g.T + b_Wg + h @ U_g.T + b_Ug)
    c = f * c_prev + i * g
    h = o * tanh(c)

Strategy: data-parallel over batch across 8 NeuronCores (1024 rows each).
Unlike the batch-on-partitions baseline, PSUM output tiles put GATE dims on
partitions and batch on the free axis (out = M_tile.T @ A_tile).  That lets
the per-gate bias ride the scalar-engine activation instruction (per-partition
bias + scale), removing all dedicated bias matmuls from the PE.

Operands are bf16 (same PE rate as f32r, half the LDWEIGHTS and DMA cost).
The last NFP8 of the 8 h-contraction subtiles run as fp8e4m3 DoubleRow
matmuls (2 K-subtiles per instruction, 2x PE throughput).  All matmul
operands on the weight side are pre-scaled by 128 on the host so the fp8
U-weights sit in e4m3's normal range; the activation instruction's
scale=1/128 undoes it before sigmoid/tanh.

Host marshalling pre-arranges every tensor into the exact SBUF layout
(partition-major), so all DMAs are contiguous per partition.
"""

import sys

sys.path.insert(0, "/opt/trn_rl_repo")

import numpy as np
import ml_dtypes

B, IN, H = 8192, 512, 1024
NCORES = 8
BLOC = B // NCORES  # 1024 batch rows per core
KTOT = IN + H  # 1536 contraction
KT = KTOT // 128  # 12 K-subtiles total
NFP8 = 6  # of the 8 h-side K-subtiles, how many run fp8 DoubleRow (even)
KBF = KT - NFP8  # leading bf16 K-subtiles (x part + first h subtiles)
NG = 4 * H // 128  # 32 gate-tiles of 128 output dims
NS = H // 128  # 8 h-slices
NCH = BLOC // 512  # 2 batch chunks of 512
SCALE = 128.0  # weight pre-scale (power of two; undone in activation)

_PROG = None  # cached so repeat calls skip rebuild/recompile


def _build_program():
    import concourse.bass as bass
    import concourse.mybir as mybir
    import concourse.tile as tile
    from concourse import bacc
    from contextlib import ExitStack

    f32 = mybir.dt.float32
    bf16 = mybir.dt.bfloat16
    f8 = mybir.dt.float8e4
    SIG = mybir.ActivationFunctionType.Sigmoid
    TANH = mybir.ActivationFunctionType.Tanh
    DR = mybir.MatmulPerfMode.DoubleRow

    nc = bacc.Bacc("TRN2", target_bir_lowering=False, debug=False)

    # Device tensors, all in final SBUF layout (partition dim first).
    # m_bf: [128, NG, KBF, 128] bf16   weights, K-subtiles 0..KBF-1
    # m_f8: [128, NG, NFP8, 128] fp8   weights, K-subtiles KBF..11
    # at_bf: [128, KBF, BLOC] bf16     activations [x | h_lo]
    # at_f8: [128, NFP8, BLOC] fp8     activations h_hi
    # bias: [128, NG] f32              per gate-tile per partition
    # bd:   [2, BLOC] bf16             boundary.T
    # wb:   [2, H] bf16                W_b.T * SCALE
    # c_in: [128, NS, BLOC] f32        c_prev.T
    # h/c out: [128, NS, BLOC] f32     transposed outputs
    m_bf_d = nc.dram_tensor("m_bf", [128, NG * KBF * 128], bf16, kind="ExternalInput").ap()
    at_bf_d = nc.dram_tensor("at_bf", [128, KBF * BLOC], bf16, kind="ExternalInput").ap()
    bias_d = nc.dram_tensor("bias_in", [128, NG], f32, kind="ExternalInput").ap()
    bd_d = nc.dram_tensor("bd_in", [2, BLOC], bf16, kind="ExternalInput").ap()
    wb_d = nc.dram_tensor("wb_in", [2, H], bf16, kind="ExternalInput").ap()
    c_d = nc.dram_tensor("c_in", [128, NS * BLOC], f32, kind="ExternalInput").ap()
    h_o = nc.dram_tensor("h_out", [128, NS * BLOC], bf16, kind="ExternalOutput").ap()
    c_o = nc.dram_tensor("c_out", [128, NS * BLOC], bf16, kind="ExternalOutput").ap()
    if NFP8:
        m_f8_d = nc.dram_tensor(
            "m_f8", [128, NG * NFP8 * 128], f8, kind="ExternalInput"
        ).ap()
        at_f8_d = nc.dram_tensor(
            "at_f8", [128, NFP8 * BLOC], f8, kind="ExternalInput"
        ).ap()

    with tile.TileContext(nc) as tc:
        with ExitStack() as ctx:
            wup = ctx.enter_context(tc.tile_pool(name="wup", bufs=1))
            cst = ctx.enter_context(tc.tile_pool(name="cst", bufs=1))
            wp = ctx.enter_context(tc.tile_pool(name="wp", bufs=1))
            actp = ctx.enter_context(tc.tile_pool(name="actp", bufs=2))
            outp = ctx.enter_context(tc.tile_pool(name="outp", bufs=4))
            psp = ctx.enter_context(tc.tile_pool(name="psp", bufs=8, space="PSUM"))

            # PE warm-up: dummy bf16 matmuls with no DMA deps push the PE
            # through its p-state ramp while the first weights load.
            wu_w = wup.tile([128, 128], bf16, name="wu_w")
            nc.vector.memset(wu_w, 0.0)
            wu_ps = psp.tile([128, 512], f32, name="wu_ps", tag="ps")
            for _ in range(72):
                nc.tensor.matmul(wu_ps[:, 0:128], wu_w, wu_w, start=True, stop=True)

            # Activations first on the sync queue (they gate the first real
            # matmuls); small constants + c_prev ride the scalar queue.
            at_bf_t = cst.tile([128, KBF, BLOC], bf16, name="at_bf_t")
            nc.sync.dma_start(
                out=at_bf_t, in_=at_bf_d[:, :].rearrange("p (k b) -> p k b", k=KBF)
            )
            if NFP8:
                at_f8_t = cst.tile([128, NFP8, BLOC], f8, name="at_f8_t")
                nc.sync.dma_start(
                    out=at_f8_t,
                    in_=at_f8_d[:, :].rearrange("p (k b) -> p k b", k=NFP8),
                )
            bias_t = cst.tile([128, NG], f32, name="bias_t")
            nc.scalar.dma_start(out=bias_t, in_=bias_d[:, :])
            bd_t = cst.tile([2, BLOC], bf16, name="bd_t")
            nc.scalar.dma_start(out=bd_t, in_=bd_d[:, :])
            wb_t = cst.tile([2, H], bf16, name="wb_t")
            nc.scalar.dma_start(out=wb_t, in_=wb_d[:, :])
            c_t = cst.tile([128, NS, BLOC], f32, name="c_t")
            nc.scalar.dma_start(
                out=c_t, in_=c_d[:, :].rearrange("p (s b) -> p s b", s=NS)
            )

            # Weights: one DMA per gate-tile, issued in compute order.
            w_bf = {}
            w_f8 = {}
            gorder = [s + 8 * z for s in range(NS) for z in range(4)]
            for g in gorder:
                t = wp.tile([128, KBF, 128], bf16, name=f"wbf{g}")
                nc.sync.dma_start(
                    out=t,
                    in_=m_bf_d[:, g * KBF * 128 : (g + 1) * KBF * 128].rearrange(
                        "p (k c) -> p k c", k=KBF
                    ),
                )
                w_bf[g] = t
                if NFP8:
                    t8 = wp.tile([128, NFP8, 128], f8, name=f"wf8{g}")
                    nc.sync.dma_start(
                        out=t8,
                        in_=m_f8_d[:, g * NFP8 * 128 : (g + 1) * NFP8 * 128].rearrange(
                            "p (k c) -> p k c", k=NFP8
                        ),
                    )
                    w_f8[g] = t8

            for ch in range(NCH):
                cs = slice(ch * 512, (ch + 1) * 512)
                for s in range(NS):
                    # gate order i, f, o, g -> gate-tile ids s, 8+s, 16+s, 24+s
                    ps = {}
                    for z in range(4):
                        g = 8 * z + s
                        p = psp.tile([128, 512], f32, name=f"ps{ch}_{s}_{z}", tag="ps")
                        ps[z] = p
                        if z == 1:
                            # boundary influence seeds the f-gate accumulator
                            nc.tensor.matmul(
                                p, wb_t[:, s * 128 : (s + 1) * 128], bd_t[:, cs],
                                start=True, stop=False,
                            )
                        for k in range(KBF):
                            nc.tensor.matmul(
                                p, w_bf[g][:, k, :], at_bf_t[:, k, cs],
                                start=(k == 0 and z != 1),
                                stop=(NFP8 == 0 and k == KBF - 1),
                            )
                        for kp in range(0, NFP8, 2):
                            nc.tensor.matmul(
                                p,
                                w_f8[g][:, kp : kp + 2, :],
                                at_f8_t[:, kp : kp + 2, cs],
                                start=False,
                                stop=(kp == NFP8 - 2),
                                perf_mode=DR,
                            )

                    # activations: sigmoid(i,f,o), tanh(g); bias+scale folded in
                    gt = {}
                    for z, fn in ((0, SIG), (1, SIG), (2, SIG), (3, TANH)):
                        g = 8 * z + s
                        t = actp.tile([128, 512], bf16, name=f"g{ch}_{s}_{z}", tag=f"g{z}")
                        nc.scalar.activation(
                            t, ps[z], fn, bias=bias_t[:, g : g + 1], scale=1.0 / SCALE
                        )
                        gt[z] = t

                    # c' = f*c + i*g ; h = o*tanh(c')  (bf16 elementwise: 2x DVE)
                    cn = outp.tile([128, 512], bf16, name=f"cn{ch}_{s}", tag="cn")
                    tmp = actp.tile([128, 512], bf16, name=f"tmp{ch}_{s}", tag="tmp")
                    nc.vector.tensor_mul(cn, gt[1], c_t[:, s, cs])
                    nc.vector.tensor_mul(tmp, gt[0], gt[3])
                    nc.vector.tensor_add(cn, cn, tmp)
                    th = actp.tile([128, 512], bf16, name=f"th{ch}_{s}", tag="th")
                    nc.scalar.activation(th, cn, TANH)
                    hn = outp.tile([128, 512], bf16, name=f"hn{ch}_{s}", tag="hn")
                    nc.vector.tensor_mul(hn, gt[2], th)

                    off = s * BLOC + ch * 512
                    nc.gpsimd.dma_start(out=c_o[:, off : off + 512], in_=cn)
                    nc.gpsimd.dma_start(out=h_o[:, off : off + 512], in_=hn)
    nc.compile()
    return nc


def _get_program():
    global _PROG
    if _PROG is None:
        _PROG = _build_program()
    return _PROG


def _prep_inputs(inputs):
    """Host-side marshalling into exact SBUF layouts (see header)."""
    f = np.float32
    bf = ml_dtypes.bfloat16
    f8 = ml_dtypes.float8_e4m3
    x = np.asarray(inputs["x"], f)
    h_prev = np.asarray(inputs["h_prev"], f)
    c_prev = np.asarray(inputs["c_prev"], f)
    boundary = np.asarray(inputs["boundary"], f)

    gates = ["i", "f", "o", "g"]
    # M [1536, 4096]: rows = contraction (x then h), cols = [i|f|o|g] x H.
    M = np.empty((KTOT, 4 * H), f)
    bias_vec = np.empty(4 * H, f)
    for zi, z in enumerate(gates):
        W = np.asarray(inputs[f"W_{z}"], f)
        U = np.asarray(inputs[f"U_{z}"], f)
        cs = slice(zi * H, (zi + 1) * H)
        M[:IN, cs] = W.T
        M[IN:, cs] = U.T
        b = np.asarray(inputs[f"b_W{z}"], f) + np.asarray(inputs[f"b_U{z}"], f)
        if z == "f":
            b = b + np.asarray(inputs["b_Wb"], f)
        bias_vec[cs] = b
    M *= SCALE

    # [K, 4H] -> [128, NG, KS, 128]: K-row = 128*kk + p, col = 128*g + c
    def dev_weights(Msub, ks):
        t = Msub.reshape(ks, 128, NG, 128).transpose(1, 2, 0, 3)
        return np.ascontiguousarray(t).reshape(128, -1)

    m_bf = dev_weights(M[: KBF * 128], KBF).astype(bf)
    bias_dev = np.ascontiguousarray(bias_vec.reshape(NG, 128).T)  # [128, NG]
    wb_dev = np.ascontiguousarray(
        (np.asarray(inputs["W_b"], f).T * SCALE).astype(bf)
    )  # [2, H]

    AT = np.concatenate([x, h_prev], axis=1).T  # [1536, 8192] (full batch)
    at_bf_full = np.ascontiguousarray(
        AT[: KBF * 128].reshape(KBF, 128, B).transpose(1, 0, 2)
    ).astype(bf)  # [128, KBF, B]
    cT = c_prev.T  # [H, B]

    in_maps = []
    if NFP8:
        m_f8 = dev_weights(M[KBF * 128 :], NFP8).astype(f8)
        at_f8_full = np.ascontiguousarray(
            AT[KBF * 128 :].reshape(NFP8, 128, B).transpose(1, 0, 2)
        ).astype(f8)  # [128, NFP8, B]
    for c in range(NCORES):
        rs = slice(c * BLOC, (c + 1) * BLOC)
        im = {
            "m_bf": m_bf,
            "at_bf": np.ascontiguousarray(at_bf_full[:, :, rs]).reshape(128, -1),
            "bias_in": bias_dev,
            "bd_in": np.ascontiguousarray(boundary[rs].T.astype(bf)),
            "wb_in": wb_dev,
            "c_in": np.ascontiguousarray(
                cT[:, rs].reshape(NS, 128, BLOC).transpose(1, 0, 2)
            ).reshape(128, -1),
        }
        if NFP8:
            im["m_f8"] = m_f8
            im["at_f8"] = np.ascontiguousarray(at_f8_full[:, :, rs]).reshape(128, -1)
        in_maps.append(im)
    return in_maps


def _unshard(res_list, key):
    """[128, NS*BLOC] bf16 per core -> [B, H] f32 full."""
    parts = []
    for r in res_list:
        t = np.asarray(r[key], dtype=np.float32)
        t = t.reshape(128, NS, BLOC).transpose(1, 0, 2).reshape(H, BLOC)
        parts.append(t.T)  # [BLOC, H]
    return np.ascontiguousarray(np.concatenate(parts, axis=0))


def run(inputs, trace=False):
    """Returns ((h, c), BassKernelResults)."""
    from concourse.bass_utils import run_bass_kernel_spmd

    nc = _get_program()
    in_maps = _prep_inputs(inputs)
    res = run_bass_kernel_spmd(
        nc, in_maps, core_ids=list(range(NCORES)), trace=trace
    )
    h = _unshard(res.results, "h_out")
    c = _unshard(res.results, "c_out")
    return (h, c), res


def kernel(**inputs):
    out, _ = run(inputs, trace=False)
    return out


# revision 10
# speedup vs baseline: 1.4831x; 1.0167x over previous
"""Trainium2 Bass kernel for a custom LSTM cell.

Math (per reference):
    i = sigmoid(x @ W_i.T + b_Wi + h @ U_i.T + b_Ui)
    f = sigmoid(x @ W_f.T + b_Wf + h @ U_f.T + b_Uf + boundary @ W_b.T + b_Wb)
    o = sigmoid(x @ W_o.T + b_Wo + h @ U_o.T + b_Uo)
    g = tanh   (x @ W_g.T + b_Wg + h @ U_g.T + b_Ug)
    c = f * c_prev + i * g
    h = o * tanh(c)

Strategy: data-parallel over batch across 8 NeuronCores (1024 rows each).
Unlike the batch-on-partitions baseline, PSUM output tiles put GATE dims on
partitions and batch on the free axis (out = M_tile.T @ A_tile).  That lets
the per-gate bias ride the scalar-engine activation instruction (per-partition
bias + scale), removing all dedicated bias matmuls from the PE.

Operands are bf16 (same PE rate as f32r, half the LDWEIGHTS and DMA cost).
The last NFP8 of the 8 h-contraction subtiles run as fp8e4m3 DoubleRow
matmuls (2 K-subtiles per instruction, 2x PE throughput).  All matmul
operands on the weight side are pre-scaled by 128 on the host so the fp8
U-weights sit in e4m3's normal range; the activation instruction's
scale=1/128 undoes it before sigmoid/tanh.

Host marshalling pre-arranges every tensor into the exact SBUF layout
(partition-major), so all DMAs are contiguous per partition.
"""

import sys

sys.path.insert(0, "/opt/trn_rl_repo")

import numpy as np
import ml_dtypes

B, IN, H = 8192, 512, 1024
NCORES = 8
BLOC = B // NCORES  # 1024 batch rows per core
KTOT = IN + H  # 1536 contraction
KT = KTOT // 128  # 12 K-subtiles total
NFP8 = 6  # of the 8 h-side K-subtiles, how many run fp8 DoubleRow (even)
KBF = KT - NFP8  # leading bf16 K-subtiles (x part + first h subtiles)
NG = 4 * H // 128  # 32 gate-tiles of 128 output dims
NS = H // 128  # 8 h-slices
NCH = BLOC // 512  # 2 batch chunks of 512
SCALE = 128.0  # weight pre-scale (power of two; undone in activation)

_PROG = None  # cached so repeat calls skip rebuild/recompile


def _build_program():
    import concourse.bass as bass
    import concourse.mybir as mybir
    import concourse.tile as tile
    from concourse import bacc
    from contextlib import ExitStack

    f32 = mybir.dt.float32
    bf16 = mybir.dt.bfloat16
    f8 = mybir.dt.float8e4
    SIG = mybir.ActivationFunctionType.Sigmoid
    TANH = mybir.ActivationFunctionType.Tanh
    DR = mybir.MatmulPerfMode.DoubleRow

    nc = bacc.Bacc("TRN2", target_bir_lowering=False, debug=False)

    # Device tensors, all in final SBUF layout (partition dim first).
    # m_bf: [128, NG, KBF, 128] bf16   weights, K-subtiles 0..KBF-1
    # m_f8: [128, NG, NFP8, 128] fp8   weights, K-subtiles KBF..11
    # at_bf: [128, KBF, BLOC] bf16     activations [x | h_lo]
    # at_f8: [128, NFP8, BLOC] fp8     activations h_hi
    # bias: [128, NG] f32              per gate-tile per partition
    # bd:   [2, BLOC] bf16             boundary.T
    # wb:   [2, H] bf16                W_b.T * SCALE
    # c_in: [128, NS, BLOC] f32        c_prev.T
    # h/c out: [128, NS, BLOC] f32     transposed outputs
    m_bf_d = nc.dram_tensor("m_bf", [128, NG * KBF * 128], bf16, kind="ExternalInput").ap()
    at_bf_d = nc.dram_tensor("at_bf", [128, KBF * BLOC], bf16, kind="ExternalInput").ap()
    bias_d = nc.dram_tensor("bias_in", [128, NG], f32, kind="ExternalInput").ap()
    bd_d = nc.dram_tensor("bd_in", [2, BLOC], bf16, kind="ExternalInput").ap()
    wb_d = nc.dram_tensor("wb_in", [2, H], bf16, kind="ExternalInput").ap()
    c_d = nc.dram_tensor("c_in", [128, NS * BLOC], f32, kind="ExternalInput").ap()
    h_o = nc.dram_tensor("h_out", [128, NS * BLOC], bf16, kind="ExternalOutput").ap()
    c_o = nc.dram_tensor("c_out", [128, NS * BLOC], bf16, kind="ExternalOutput").ap()
    if NFP8:
        m_f8_d = nc.dram_tensor(
            "m_f8", [128, NG * NFP8 * 128], f8, kind="ExternalInput"
        ).ap()
        at_f8_d = nc.dram_tensor(
            "at_f8", [128, NFP8 * BLOC], f8, kind="ExternalInput"
        ).ap()

    with tile.TileContext(nc) as tc:
        with ExitStack() as ctx:
            wup = ctx.enter_context(tc.tile_pool(name="wup", bufs=1))
            cst = ctx.enter_context(tc.tile_pool(name="cst", bufs=1))
            wp = ctx.enter_context(tc.tile_pool(name="wp", bufs=1))
            actp = ctx.enter_context(tc.tile_pool(name="actp", bufs=2))
            outp = ctx.enter_context(tc.tile_pool(name="outp", bufs=4))
            psp = ctx.enter_context(tc.tile_pool(name="psp", bufs=8, space="PSUM"))

            # PE warm-up: dummy bf16 matmuls with no DMA deps push the PE
            # through its p-state ramp while the first weights load.
            wu_w = wup.tile([128, 128], bf16, name="wu_w")
            nc.vector.memset(wu_w, 0.0)
            wu_ps = psp.tile([128, 512], f32, name="wu_ps", tag="ps")
            for _ in range(48):
                nc.tensor.matmul(wu_ps[:, 0:128], wu_w, wu_w, start=True, stop=True)

            # Small constants + c_prev ride the scalar queue.
            bias_t = cst.tile([128, NG], f32, name="bias_t")
            nc.scalar.dma_start(out=bias_t, in_=bias_d[:, :])
            bd_t = cst.tile([2, BLOC], bf16, name="bd_t")
            nc.scalar.dma_start(out=bd_t, in_=bd_d[:, :])
            wb_t = cst.tile([2, H], bf16, name="wb_t")
            nc.scalar.dma_start(out=wb_t, in_=wb_d[:, :])
            c_t = cst.tile([128, NS, BLOC], f32, name="c_t")
            nc.scalar.dma_start(
                out=c_t, in_=c_d[:, :].rearrange("p (s b) -> p s b", s=NS)
            )

            # Bulk data on the sync queue, ordered so the PE can start as soon
            # as possible: first weight tiles for gate-tile group 0, then the
            # activation K-subtiles, then the rest of the weights (grouped 4
            # gate-tiles per DMA for fat descriptors).  Region-level dependency
            # tracking lets matmuls start as their slices land.
            at_bf_t = cst.tile([128, KBF, BLOC], bf16, name="at_bf_t")
            at_f8_t = cst.tile([128, NFP8, BLOC], f8, name="at_f8_t") if NFP8 else None
            # weight storage: one merged tile per 4 consecutive gate-tiles in
            # compute order; w_bf[g] = (tile, idx-in-group)
            gorder = [s + 8 * z for s in range(NS) for z in range(4)]
            w_bf = {}
            w_f8 = {}
            wtiles = []
            for gi in range(0, NG, 4):
                grp = gorder[gi : gi + 4]
                t = wp.tile([128, 4, KBF, 128], bf16, name=f"wbf_g{gi}")
                t8 = wp.tile([128, 4, NFP8, 128], f8, name=f"wf8_g{gi}") if NFP8 else None
                for j, g in enumerate(grp):
                    w_bf[g] = t[:, j]
                    if NFP8:
                        w_f8[g] = t8[:, j]
                wtiles.append((grp, t, t8))

            def load_wgrp(grp, t, t8):
                for j, g in enumerate(grp):
                    nc.sync.dma_start(
                        out=t[:, j],
                        in_=m_bf_d[:, g * KBF * 128 : (g + 1) * KBF * 128].rearrange(
                            "p (k c) -> p k c", k=KBF
                        ),
                    )
                    if NFP8:
                        nc.sync.dma_start(
                            out=t8[:, j],
                            in_=m_f8_d[
                                :, g * NFP8 * 128 : (g + 1) * NFP8 * 128
                            ].rearrange("p (k c) -> p k c", k=NFP8),
                        )

            # group 0 weights first (PE's first matmuls), then activations
            # k-by-k, then remaining weight groups
            load_wgrp(*wtiles[0])
            for k in range(KBF):
                nc.sync.dma_start(
                    out=at_bf_t[:, k],
                    in_=at_bf_d[:, k * BLOC : (k + 1) * BLOC],
                )
            for k in range(NFP8):
                nc.sync.dma_start(
                    out=at_f8_t[:, k],
                    in_=at_f8_d[:, k * BLOC : (k + 1) * BLOC],
                )
            for grp, t, t8 in wtiles[1:]:
                load_wgrp(grp, t, t8)

            for ch in range(NCH):
                cs = slice(ch * 512, (ch + 1) * 512)
                for s in range(NS):
                    # gate order i, f, o, g -> gate-tile ids s, 8+s, 16+s, 24+s
                    ps = {}
                    for z in range(4):
                        g = 8 * z + s
                        p = psp.tile([128, 512], f32, name=f"ps{ch}_{s}_{z}", tag="ps")
                        ps[z] = p
                        if z == 1:
                            # boundary influence seeds the f-gate accumulator
                            nc.tensor.matmul(
                                p, wb_t[:, s * 128 : (s + 1) * 128], bd_t[:, cs],
                                start=True, stop=False,
                            )
                        for k in range(KBF):
                            nc.tensor.matmul(
                                p, w_bf[g][:, k, :], at_bf_t[:, k, cs],
                                start=(k == 0 and z != 1),
                                stop=(NFP8 == 0 and k == KBF - 1),
                            )
                        for kp in range(0, NFP8, 2):
                            nc.tensor.matmul(
                                p,
                                w_f8[g][:, kp : kp + 2, :],
                                at_f8_t[:, kp : kp + 2, cs],
                                start=False,
                                stop=(kp == NFP8 - 2),
                                perf_mode=DR,
                            )

                    # activations: sigmoid(i,f,o), tanh(g); bias+scale folded in
                    gt = {}
                    for z, fn in ((0, SIG), (1, SIG), (2, SIG), (3, TANH)):
                        g = 8 * z + s
                        t = actp.tile([128, 512], bf16, name=f"g{ch}_{s}_{z}", tag=f"g{z}")
                        nc.scalar.activation(
                            t, ps[z], fn, bias=bias_t[:, g : g + 1], scale=1.0 / SCALE
                        )
                        gt[z] = t

                    # c' = f*c + i*g ; h = o*tanh(c')  (bf16 elementwise: 2x DVE)
                    cn = outp.tile([128, 512], bf16, name=f"cn{ch}_{s}", tag="cn")
                    tmp = actp.tile([128, 512], bf16, name=f"tmp{ch}_{s}", tag="tmp")
                    nc.vector.tensor_mul(cn, gt[1], c_t[:, s, cs])
                    nc.vector.tensor_mul(tmp, gt[0], gt[3])
                    nc.vector.tensor_add(cn, cn, tmp)
                    th = actp.tile([128, 512], bf16, name=f"th{ch}_{s}", tag="th")
                    nc.scalar.activation(th, cn, TANH)
                    hn = outp.tile([128, 512], bf16, name=f"hn{ch}_{s}", tag="hn")
                    nc.vector.tensor_mul(hn, gt[2], th)

                    off = s * BLOC + ch * 512
                    nc.gpsimd.dma_start(out=c_o[:, off : off + 512], in_=cn)
                    nc.gpsimd.dma_start(out=h_o[:, off : off + 512], in_=hn)
    nc.compile()
    return nc


def _get_program():
    global _PROG
    if _PROG is None:
        _PROG = _build_program()
    return _PROG


def _prep_inputs(inputs):
    """Host-side marshalling into exact SBUF layouts (see header)."""
    f = np.float32
    bf = ml_dtypes.bfloat16
    f8 = ml_dtypes.float8_e4m3
    x = np.asarray(inputs["x"], f)
    h_prev = np.asarray(inputs["h_prev"], f)
    c_prev = np.asarray(inputs["c_prev"], f)
    boundary = np.asarray(inputs["boundary"], f)

    gates = ["i", "f", "o", "g"]
    # M [1536, 4096]: rows = contraction (x then h), cols = [i|f|o|g] x H.
    M = np.empty((KTOT, 4 * H), f)
    bias_vec = np.empty(4 * H, f)
    for zi, z in enumerate(gates):
        W = np.asarray(inputs[f"W_{z}"], f)
        U = np.asarray(inputs[f"U_{z}"], f)
        cs = slice(zi * H, (zi + 1) * H)
        M[:IN, cs] = W.T
        M[IN:, cs] = U.T
        b = np.asarray(inputs[f"b_W{z}"], f) + np.asarray(inputs[f"b_U{z}"], f)
        if z == "f":
            b = b + np.asarray(inputs["b_Wb"], f)
        bias_vec[cs] = b
    M *= SCALE

    # [K, 4H] -> [128, NG, KS, 128]: K-row = 128*kk + p, col = 128*g + c
    def dev_weights(Msub, ks):
        t = Msub.reshape(ks, 128, NG, 128).transpose(1, 2, 0, 3)
        return np.ascontiguousarray(t).reshape(128, -1)

    m_bf = dev_weights(M[: KBF * 128], KBF).astype(bf)
    bias_dev = np.ascontiguousarray(bias_vec.reshape(NG, 128).T)  # [128, NG]
    wb_dev = np.ascontiguousarray(
        (np.asarray(inputs["W_b"], f).T * SCALE).astype(bf)
    )  # [2, H]

    AT = np.concatenate([x, h_prev], axis=1).T  # [1536, 8192] (full batch)
    at_bf_full = np.ascontiguousarray(
        AT[: KBF * 128].reshape(KBF, 128, B).transpose(1, 0, 2)
    ).astype(bf)  # [128, KBF, B]
    cT = c_prev.T  # [H, B]

    in_maps = []
    if NFP8:
        m_f8 = dev_weights(M[KBF * 128 :], NFP8).astype(f8)
        at_f8_full = np.ascontiguousarray(
            AT[KBF * 128 :].reshape(NFP8, 128, B).transpose(1, 0, 2)
        ).astype(f8)  # [128, NFP8, B]
    for c in range(NCORES):
        rs = slice(c * BLOC, (c + 1) * BLOC)
        im = {
            "m_bf": m_bf,
            "at_bf": np.ascontiguousarray(at_bf_full[:, :, rs]).reshape(128, -1),
            "bias_in": bias_dev,
            "bd_in": np.ascontiguousarray(boundary[rs].T.astype(bf)),
            "wb_in": wb_dev,
            "c_in": np.ascontiguousarray(
                cT[:, rs].reshape(NS, 128, BLOC).transpose(1, 0, 2)
            ).reshape(128, -1),
        }
        if NFP8:
            im["m_f8"] = m_f8
            im["at_f8"] = np.ascontiguousarray(at_f8_full[:, :, rs]).reshape(128, -1)
        in_maps.append(im)
    return in_maps


def _unshard(res_list, key):
    """[128, NS*BLOC] bf16 per core -> [B, H] f32 full."""
    parts = []
    for r in res_list:
        t = np.asarray(r[key], dtype=np.float32)
        t = t.reshape(128, NS, BLOC).transpose(1, 0, 2).reshape(H, BLOC)
        parts.append(t.T)  # [BLOC, H]
    return np.ascontiguousarray(np.concatenate(parts, axis=0))


def run(inputs, trace=False):
    """Returns ((h, c), BassKernelResults)."""
    from concourse.bass_utils import run_bass_kernel_spmd

    nc = _get_program()
    in_maps = _prep_inputs(inputs)
    res = run_bass_kernel_spmd(
        nc, in_maps, core_ids=list(range(NCORES)), trace=trace
    )
    h = _unshard(res.results, "h_out")
    c = _unshard(res.results, "c_out")
    return (h, c), res


def kernel(**inputs):
    out, _ = run(inputs, trace=False)
    return out
